# revision 7
# baseline (speedup 1.0000x reference)
"""CrossScaleAttention Trainium2 kernel.

Full (unsharded) contract: kernel(query, key, value) with shapes
  query/key/value: (4, 4096, 256) float32  ->  out (4, 4096, 256) float32

reference math:
  q = l2norm(query); k = l2norm(key)
  out = softmax((q @ k^T) * 32**-0.5) @ value

Sharding: 8 cores; core c computes batch c//2, query rows (c%2)*2048..+2048,
with that batch's full K/V resident per core (no collectives).

Per-core algorithm (all matmuls contract over the partition dim):
  - normalize K and Q rows in natural layout, PE-transpose into
    K^T/Q^T [d=128x2, tokens] so the QK matmul can contract over d.
  - S^T chunks [128 keys, 512 queries] on PE; exp(S * scale) via ACT
    directly PSUM->SBUF producing P^T; no max-subtraction needed since
    scores are cosine similarities scaled by 0.177 (|logit| <= 0.177).
  - AV: out_psum[128 q, 257] += P^T_chunk.T @ [V | 1]; the appended ones
    column accumulates the softmax denominator in the same matmul chain.
  - epilogue: out = out_psum[:, :256] * (1 / out_psum[:, 256]).
"""

import sys

if "/opt/trn_rl_repo" not in sys.path:
    sys.path.insert(0, "/opt/trn_rl_repo")

import numpy as np

import concourse.bass as bass
import concourse.mybir as mybir
import concourse.tile as tile
from concourse import bacc
from concourse.bass_utils import run_bass_kernel_spmd
from concourse.masks import make_identity

F32 = mybir.dt.float32
F32R = mybir.dt.float32r
BF16 = mybir.dt.bfloat16

# dtype knobs for the two matmul stages (float32r = full-rate fp32 PE mode)
QK_DT = F32R
AV_DT = F32R


def _storage(dt):
    # fp32r matmul operands must be produced pre-rounded (BIR verifier):
    # the tiles are genuinely float32r; DVE/ACT producers do the rounding.
    return dt


def _mm(ap, dt):
    return ap

B, NQ_FULL, NK, D = 4, 4096, 4096, 256
N_CORES = 8
NQ = NQ_FULL * B // N_CORES  # 2048 queries per core
P = 128
DC = D // P          # 2 d-chunks
KC = NK // P         # 32 key chunks
QB = 512             # queries per block
NB = NQ // QB        # 4 blocks
QT = QB // P         # 4 q-subtiles per block
SCALE = float(D // 8) ** -0.5  # head_dim**-0.5 = 32**-0.5


def _build_program():
    nc = bacc.Bacc(
        "TRN2",
        target_bir_lowering=False,
        debug=False,
        enable_asserts=False,
        num_devices=N_CORES,
    )
    q_d = nc.dram_tensor("q", (NQ, D), F32, kind="ExternalInput").ap()
    k_d = nc.dram_tensor("k", (NK, D), F32, kind="ExternalInput").ap()
    v_d = nc.dram_tensor("v", (NK, D), F32, kind="ExternalInput").ap()
    o_d = nc.dram_tensor("o", (NQ, D), F32, kind="ExternalOutput").ap()

    with tile.TileContext(nc) as tc:
        with (
            tc.tile_pool(name="const", bufs=1) as const_pool,
            tc.tile_pool(name="persist", bufs=1) as persist,
            tc.tile_pool(name="loads", bufs=3) as loads,
            tc.tile_pool(name="small", bufs=6) as small,
            tc.tile_pool(name="pt", bufs=4) as pt_pool,
            tc.tile_pool(name="outs", bufs=3) as out_pool,
            tc.tile_pool(name="ps", bufs=3, space="PSUM") as ps_pool,
            tc.tile_pool(name="avps", bufs=1, space="PSUM") as av_pool,
        ):
            ident = const_pool.tile([P, P], F32)
            make_identity(nc, ident)

            # persistent operands
            kt = persist.tile([P, DC, NK], _storage(QK_DT))     # K^T: [d, keys]
            qt = persist.tile([P, DC, NQ], _storage(QK_DT))     # Q^T: [d, queries]
            # [keys, d | ones, ones]: fp32r matmuls need an even moving-dim,
            # so pad to D+2; both extra columns are 1.0 (col D = softmax denom)
            VW = D + 2
            va = persist.tile([P, KC, VW], _storage(AV_DT))
            ones = const_pool.tile([P, 1], F32)
            nc.vector.memset(ones, 1.0)
            nc.vector.tensor_copy(
                va[:, :, D:VW], ones[:, :, None].to_broadcast((P, KC, 2))
            )

            def normalize_transpose(src_d, n_tiles, dst):
                """Load [128,256] natural tiles, l2-normalize rows, transpose
                into dst[:, dc, i*128:(i+1)*128]."""
                for i in range(n_tiles):
                    nat = loads.tile([P, D], F32, tag="nat")
                    nc.sync.dma_start(nat, src_d[i * P : (i + 1) * P, :])
                    sq = loads.tile([P, D], F32, tag="sq")
                    nc.vector.tensor_mul(sq, nat, nat)
                    ss = small.tile([P, 1], F32, tag="ss")
                    nc.vector.tensor_reduce(
                        ss, sq, axis=mybir.AxisListType.X, op=mybir.AluOpType.add
                    )
                    nrm = small.tile([P, 1], F32, tag="nrm")
                    nc.scalar.sqrt(nrm, ss)
                    rinv = small.tile([P, 1], F32, tag="rinv")
                    nc.vector.reciprocal(rinv, nrm)
                    xn = loads.tile([P, D], F32, tag="xn")
                    nc.vector.tensor_scalar_mul(xn, nat, rinv)
                    for dc in range(DC):
                        tps = ps_pool.tile([P, 4 * P], F32, tag="st")
                        nc.tensor.transpose(
                            tps[:, :P], xn[:, dc * P : (dc + 1) * P], ident
                        )
                        nc.vector.tensor_copy(dst[:, dc, i * P : (i + 1) * P], tps[:, :P])

            normalize_transpose(k_d, KC, kt)
            normalize_transpose(q_d, NQ // P, qt)

            # V loads (dtype-converted if needed)
            for i in range(KC):
                if AV_DT == F32:
                    nc.sync.dma_start(va[:, i, :D], v_d[i * P : (i + 1) * P, :])
                else:
                    vload = loads.tile([P, D], F32, tag="nat")
                    nc.sync.dma_start(vload, v_d[i * P : (i + 1) * P, :])
                    nc.vector.tensor_copy(va[:, i, :D], vload)

            # main loop
            for blk in range(NB):
                avs = [
                    av_pool.tile([P, D + 2], F32, tag=f"av{t}", name=f"av{t}_{blk}")
                    for t in range(QT)
                ]
                for kk in range(KC):
                    st = ps_pool.tile([P, QB], F32, tag="st", name=f"st_{blk}_{kk}")
                    for dc in range(DC):
                        nc.tensor.matmul(
                            st,
                            lhsT=_mm(kt[:, dc, kk * P : (kk + 1) * P], QK_DT),
                            rhs=_mm(qt[:, dc, blk * QB : (blk + 1) * QB], QK_DT),
                            start=(dc == 0),
                            stop=(dc == DC - 1),
                        )
                    pt = pt_pool.tile([P, QB], _storage(AV_DT), tag="pt", name=f"pt_{blk}_{kk}")
                    nc.scalar.activation(
                        pt, st, mybir.ActivationFunctionType.Exp, scale=SCALE
                    )
                    for t in range(QT):
                        nc.tensor.matmul(
                            avs[t],
                            lhsT=_mm(pt[:, t * P : (t + 1) * P], AV_DT),
                            rhs=_mm(va[:, kk, :], AV_DT),
                            start=(kk == 0),
                            stop=(kk == KC - 1),
                        )
                for t in range(QT):
                    rec = small.tile([P, 1], F32, tag="rec")
                    nc.vector.reciprocal(rec, avs[t][:, D : D + 1])
                    ot = out_pool.tile([P, D], F32, tag="ot")
                    nc.vector.tensor_scalar_mul(ot, avs[t][:, :D], rec)
                    row = blk * QB + t * P
                    nc.sync.dma_start(o_d[row : row + P, :], ot)

    nc.compile()
    return nc


_CACHED = {}


def _get_program():
    if "nc" not in _CACHED:
        _CACHED["nc"] = _build_program()
    return _CACHED["nc"]


def _make_in_maps(query, key, value):
    in_maps = []
    for c in range(N_CORES):
        b = c // (N_CORES // B)
        qs = (c % (N_CORES // B)) * NQ
        in_maps.append(
            {
                "q": np.ascontiguousarray(query[b, qs : qs + NQ], dtype=np.float32),
                "k": np.ascontiguousarray(key[b], dtype=np.float32),
                "v": np.ascontiguousarray(value[b], dtype=np.float32),
            }
        )
    return in_maps


def run_sharded(query, key, value, trace=False):
    """Returns (out, BassKernelResults)."""
    nc = _get_program()
    in_maps = _make_in_maps(query, key, value)
    res = run_bass_kernel_spmd(nc, in_maps, core_ids=list(range(N_CORES)), trace=trace)
    out = np.empty((B, NQ_FULL, D), dtype=np.float32)
    for c in range(N_CORES):
        b = c // (N_CORES // B)
        qs = (c % (N_CORES // B)) * NQ
        out[b, qs : qs + NQ] = res.results[c]["o"]
    return out, res


def kernel(query, key, value):
    query = np.asarray(query)
    key = np.asarray(key)
    value = np.asarray(value)
    out, _ = run_sharded(query, key, value)
    return out
